# revision 18
# baseline (speedup 1.0000x reference)
"""Trainium2 Bass kernel for Controller.predict_pairwise_prob (cumm='sum').

Math (per batch b, with T=512 timesteps, C=32 channels):
    a   = log(coref + overwrite)                       [T, C]
    bb  = log(coref)                                   [T, C]
    cum = cumsum_t log((1-overwrite)*(1-EPS) + EPS)    [T, C]
    out[t1, t2] = logsumexp_c(a[t1] + bb[t2] + cum[t2] - cum[t1]) * (t2 > t1)

Reformulation used here: let cum0[t] = cum[t, 0] and cumd = cum - cum0
(per-t channel-0 shift; channel spread of cum stays within ~+-25 so exp
stays in fp32 range while the +-360 growth of cum cancels):

    u2 = a - cumd, v2 = bb + cumd           (both in [-31, 25])
    S[t1, t2] = sum_c exp(u2[t1, c]) * exp(v2[t2, c])
    out = ln S - cum0[t1] + cum0[t2]

Layout: everything stays t-major ([t(128 partitions), block, chan]) until
exp time, so all elementwise ops run 128 lanes wide with 128-elem free
dims.  The cumsum is a triangular matmul per 128-block plus a tiny prefix
fix-up across the 4 blocks.  u2|v2|cum0 (65 cols, fp16) are transposed
per block on the PE; exp produces bf16 [64, 512] operands for the K=32
pairwise matmuls.  Constants (identity, triangular U, mask, ones) are
built on-device with memset+affine_select - no DRAM constants at all.

Sharding: data-parallel over batch, one batch element per NeuronCore.
"""

import numpy as np

import concourse.bacc as bacc
import concourse.tile as tile
from concourse import mybir
from concourse.bass_utils import run_bass_kernel_spmd

EPS = 1e-8
P = 128          # partitions / t-block size
T = 512          # timesteps
C = 32           # channels
NB = T // P      # 4 t-blocks
FP = mybir.dt.float32
FR = mybir.dt.float32r
F16 = mybir.dt.float16
BF = mybir.dt.bfloat16
ALU = mybir.AluOpType
AF = mybir.ActivationFunctionType

_CACHE = {}


def _build():
    import concourse.bacc as _bacc_mod
    import concourse.hw_specs as _hw

    _orig_tables = _hw.get_activation_tables
    _only = "natural_log_exp_and_others"

    def _patched(arch):
        tabs = _orig_tables(arch)
        return {k: (v if k == _only else set()) for k, v in tabs.items()}

    _bacc_mod.get_activation_tables = _patched
    nc = bacc.Bacc(
        "TRN2",
        target_bir_lowering=False,
        debug=False,
        enable_asserts=False,
        num_devices=8,
    )

    corow = nc.dram_tensor("corow", [T, 2 * C], FP, kind="ExternalInput").ap()
    out = nc.dram_tensor("out", [T, T], FP, kind="ExternalOutput").ap()

    with tile.TileContext(nc) as tc:
        _body(tc, out, corow)

    nc.compile()
    return nc


def _body(tc, out, corow):
    nc = tc.nc
    with (
        tc.tile_pool(name="main", bufs=1) as pool,
        tc.tile_pool(name="pp", bufs=4) as pp,
        tc.tile_pool(name="ps", bufs=1, space="PSUM") as psum,
        tc.tile_pool(name="ps_s", bufs=4, space="PSUM") as psum_s,
    ):
        # ---- input DMAs first: ow half on sync (gates wf -> cumsum chain),
        # cor half on gpsimd; separate tiles so wf only waits on ow ----
        src = corow.rearrange("(n p) x -> p n x", p=P)
        pkow = pool.tile([P, NB, C], FP, tag="pkow")
        nc.sync.dma_start(pkow[:], src[:, :, C:2 * C])
        pkcor = pool.tile([P, NB, C], FP, tag="pkcor")
        nc.gpsimd.dma_start(pkcor[:], src[:, :, 0:C])

        # ---- on-device constants (run during the DMA wait) ----
        tri = [[1, P]]  # iota[p, f] = f - p with channel_multiplier=-1
        identh = pool.tile([P, P], F16, tag="identh")
        nc.gpsimd.memset(identh[:], 1.0)
        nc.gpsimd.affine_select(identh[:], identh[:], tri, ALU.is_equal, 0.0,
                                base=0, channel_multiplier=-1)
        u16 = pool.tile([P, P], F16, tag="u16")
        nc.gpsimd.memset(u16[:], 1.0)
        nc.gpsimd.affine_select(u16[:], u16[:], tri, ALU.is_ge, 0.0,
                                base=0, channel_multiplier=-1)
        maskf = pool.tile([P, P], FP, tag="maskf")
        nc.gpsimd.memset(maskf[:], 1.0)
        nc.gpsimd.affine_select(maskf[:], maskf[:], tri, ALU.is_gt, 0.0,
                                base=0, channel_multiplier=-1)
        onescol = pool.tile([P, 1], F16, tag="onescol")
        nc.vector.memset(onescol[:], 1.0)
        ones16 = pool.tile([1, P], F16, tag="ones16")
        nc.vector.memset(ones16[:], 1.0)


        # ---- elementwise logs, t-major (128 lanes, 128-free ops) ----
        ts = pool.tile([P, NB, C], FP, tag="ts")
        nc.vector.tensor_add(ts[:], pkcor[:], pkow[:])
        wf = pool.tile([P, NB, C], F16, tag="wf")
        nc.scalar.activation(wf[:], pkow[:], AF.Ln,
                             bias=1.0, scale=-(1.0 - EPS))
        a_t = pool.tile([P, NB, C], FP, tag="a")
        nc.scalar.activation(a_t[:], ts[:], AF.Ln)
        b_t = pool.tile([P, NB, C], FP, tag="b")
        nc.scalar.activation(b_t[:], pkcor[:], AF.Ln)

        # ---- block totals, exclusive prefix, folded into wf row 0 so ONE
        # triangular matmul emits the full 512-length cumsum ----
        cum_ps = psum.tile([P, 128 + NB * C], FP, tag="cum_ps")
        nc.tensor.matmul(cum_ps[0:1, 128:128 + NB * C], onescol[:],
                         wf[:].rearrange("p n c -> p (n c)"), start=True, stop=True)
        tots = cum_ps[0:1, 128:128 + NB * C].rearrange("p (n c) -> p n c", c=C)
        totsb = pool.tile([1, NB, C], FP, tag="totsb")
        nc.vector.tensor_copy(totsb[:], tots[:])
        offs = pool.tile([1, 3, C], FP, tag="offs")
        nc.vector.tensor_copy(offs[:, 0, :], totsb[:, 0, :])
        nc.vector.tensor_add(offs[:, 1:3, :], totsb[:, 1:3, :], totsb[:, 0:2, :])
        nc.vector.tensor_add(offs[:, 2, :], offs[:, 2, :], totsb[:, 0, :])
        nc.vector.tensor_add(wf[0:1, 1:NB, :], wf[0:1, 1:NB, :], offs[:])
        cumb = cum_ps[:, 0:128].rearrange("p (n c) -> p n c", c=C)
        nc.tensor.matmul(cum_ps[:, 0:128], u16[:],
                         wf[:].rearrange("p n c -> p (n c)"), start=True, stop=True)
        cum0sb = pool.tile([P, NB, 1], FP, tag="cum0sb")
        nc.vector.tensor_copy(cum0sb[:], cumb[:, :, 0:1])

        # ---- cumd = cum - cum0 (per-t channel-0 shift), then u2|v2|cum0
        # fp16 transpose stripes ----
        cumd = pool.tile([P, NB, C], FP, tag="cumd")
        for n in range(NB):
            nc.vector.tensor_scalar_sub(cumd[:, n, :], cumb[:, n, :],
                                        cum0sb[:, n, 0:1])
        uv = pool.tile([P, NB, 2 * C + 1], F16, tag="uv")
        nc.vector.tensor_sub(uv[:, :, 0:C], a_t[:], cumd[:])
        nc.gpsimd.tensor_add(uv[:, :, C:2 * C], b_t[:], cumd[:])
        nc.vector.tensor_copy(uv[:, :, 2 * C:2 * C + 1], cum0sb[:])

        # ---- transpose each block: [128, 65] -> [65, 128] ----
        tps = psum.tile([2 * C + 1, T], F16, tag="tps")
        for n in range(NB):
            nc.tensor.transpose(tps[:, P * n:P * (n + 1)], uv[:, n, :], identh[:])

        # ---- exp into bf16 matmul operands; cum0 row for the t2 shift ----
        vht = pool.tile([C, T], BF, tag="vht")
        nc.scalar.activation(vht[:], tps[C:2 * C, :], AF.Exp)
        uht = pool.tile([C, T], BF, tag="uht")
        nc.scalar.activation(uht[:, 0:P], tps[0:C, 0:P], AF.Exp)
        nc.scalar.activation(uht[:, P:], tps[0:C, P:], AF.Exp)
        s2row = pool.tile([1, T], F16, tag="s2row")
        nc.vector.tensor_copy(s2row[:], tps[2 * C:2 * C + 1, :])

        s2bc = psum.tile([P, T], FP, tag="s2bc")
        first = True
        for i in range(NB):
            lo = P * i
            s_ps = psum_s.tile([P, T], FP, tag="s")
            nc.tensor.matmul(s_ps[:, lo:], uht[:, lo:lo + P], vht[:, lo:],
                             start=True, stop=True)
            if first:
                # slot the s2 broadcast right after the first pairwise matmul
                nc.tensor.matmul(s2bc[:], ones16[:], s2row[:], start=True, stop=True)
                first = False
            lns = pp.tile([P, T], FP, tag="lns")
            nc.scalar.activation(lns[:, lo:], s_ps[:, lo:], AF.Ln)
            o_t = pp.tile([P, T], FP, tag="o")
            nc.vector.scalar_tensor_tensor(
                out=o_t[:, lo:], in0=lns[:, lo:], scalar=cum0sb[:, i, 0:1],
                in1=s2bc[:, lo:], op0=ALU.subtract, op1=ALU.add)
            nc.gpsimd.tensor_mul(o_t[:, lo:lo + P], o_t[:, lo:lo + P], maskf[:])
            dmae = nc.sync if i % 2 == 0 else nc.scalar
            dmae.dma_start(out[lo:lo + P, lo:], o_t[:, lo:])


def kernel(coref: np.ndarray, overwrite: np.ndarray) -> np.ndarray:
    B = coref.shape[0]
    assert coref.shape == (B, T, C) and overwrite.shape == (B, T, C)
    if "nc" not in _CACHE:
        _CACHE["nc"] = _build()
    nc = _CACHE["nc"]
    in_maps = [
        {
            "corow": np.concatenate(
                [
                    np.ascontiguousarray(coref[b], dtype=np.float32),
                    np.ascontiguousarray(overwrite[b], dtype=np.float32),
                ],
                axis=1,
            ),
        }
        for b in range(B)
    ]
    res = run_bass_kernel_spmd(nc, in_maps, core_ids=list(range(B)))
    return np.stack([r["out"] for r in res.results], axis=0)
